# revision 10
# baseline (speedup 1.0000x reference)
"""Trainium2 Bass kernel for nn_DependencyParser (BiLSTM + biaffine-style scorer).

Strategy: batch-parallel over 8 NeuronCores (2 batch rows per core), zero
cross-core communication.  Per core:
  - embedding gather via indirect DMA (word table) + one-hot matmul (tag table)
  - 2-layer BiLSTM with transposed state layout: gates live as [128(H), cols]
    tiles; the per-step input contribution U = Wih^T x (+bias) is precomputed
    into SBUF with columns interleaved (t, gate, b) so each step's gate
    nonlinearities are two contiguous ACT instructions; the recurrent matmul
    writes a fresh [128, 8] PSUM tile each step (4 gate-chunk matmuls).
  - pairwise scorer: aT/cT = W1/W2 @ h in [100(k), token] layout; for each
    (batch row, 8-row i-block): one broadcast-AP DVE add builds
    tanh-input [100, 8*256], one ACT tanh in place, then fc2 contraction as
    M=1 matvecs (static weights, +bias via a constant ones row) packed into
    PSUM partitions {0,32,64,96}, copied out and DMA'd straight into the
    [Bs, L, L] output layout.

kernel(**inputs) accepts the full unsharded inputs and returns [L, B, L, 1].
"""
import numpy as np

import concourse.bass as bass
import concourse.bacc as bacc
import concourse.tile as tile
from concourse import mybir, bass_utils
from concourse.masks import make_identity

F32 = mybir.dt.float32
I32 = mybir.dt.int32
AF = mybir.ActivationFunctionType
OP = mybir.AluOpType

B, L, H, D = 16, 256, 128, 128
WE, PE_DIM, TV, TT = 100, 28, 32000, 50
NCORES = 8
Bs = B // NCORES          # 2
TOK = L * Bs              # 512
GATE_ORDER = [0, 1, 3, 2]  # pytorch [i,f,g,o] blocks -> [i,f,o,g]
GBLK = 8                  # scorer i-block size

_CACHE = {}


def _reorder_rows(w):
    return np.concatenate([w[g * H:(g + 1) * H] for g in GATE_ORDER], 0)


def _dir_weights(wih, whh, bih, bhh):
    wr = _reorder_rows(np.asarray(wih, np.float32))
    hr = _reorder_rows(np.asarray(whh, np.float32))
    br = _reorder_rows((np.asarray(bih, np.float32) + np.asarray(bhh, np.float32))[:, None])[:, 0]
    return (np.ascontiguousarray(wr.T), np.ascontiguousarray(hr.T),
            np.ascontiguousarray(br.reshape(4, H).T))


def _build(l=L):
    tok = l * Bs
    nblk = tok // 128
    nc = bacc.Bacc("TRN2", num_devices=NCORES)
    dt = nc.dram_tensor
    d_widx = dt("widx", [128, nblk], I32, kind="ExternalInput").ap()
    d_pidx = dt("pidx", [1, tok], F32, kind="ExternalInput").ap()
    d_wemb = dt("wemb", [TV, WE], F32, kind="ExternalInput").ap()
    d_temb = dt("temb", [TT, PE_DIM], F32, kind="ExternalInput").ap()
    d_wih0 = dt("wih0", [D, 2, 4 * H], F32, kind="ExternalInput").ap()
    d_whh0 = dt("whh0", [H, 2, 4 * H], F32, kind="ExternalInput").ap()
    d_b0 = dt("b0", [H, 2, 4], F32, kind="ExternalInput").ap()
    d_wih1 = dt("wih1", [H, 2, 2, 4 * H], F32, kind="ExternalInput").ap()
    d_whh1 = dt("whh1", [H, 2, 4 * H], F32, kind="ExternalInput").ap()
    d_b1 = dt("b1", [H, 2, 4], F32, kind="ExternalInput").ap()
    d_w1t = dt("w1t", [H, 2, 100], F32, kind="ExternalInput").ap()
    d_w2t = dt("w2t", [H, 2, 100], F32, kind="ExternalInput").ap()
    d_fc1b = dt("fc1b", [100, 1], F32, kind="ExternalInput").ap()
    d_w2aug = dt("w2aug", [101, 1], F32, kind="ExternalInput").ap()
    d_out = dt("scores", [Bs, l, l], F32, kind="ExternalOutput").ap()

    with tile.TileContext(nc) as tc:
        _emit(nc, tc, l, tok, nblk, d_widx, d_pidx, d_wemb, d_temb,
              d_wih0, d_whh0, d_b0, d_wih1, d_whh1, d_b1,
              d_w1t, d_w2t, d_fc1b, d_w2aug, d_out)
    nc.compile()
    return nc


def _emit(nc, tc, l, tok, nblk, d_widx, d_pidx, d_wemb, d_temb,
          d_wih0, d_whh0, d_b0, d_wih1, d_whh1, d_b1,
          d_w1t, d_w2t, d_fc1b, d_w2aug, d_out):
    import contextlib
    ctx = contextlib.ExitStack()
    cn = ctx.enter_context(tc.tile_pool(name="const", bufs=1))
    wk = ctx.enter_context(tc.tile_pool(name="work", bufs=1))


    # ---- load constants -------------------------------------------------
    def load(name, dram, shape=None, rows=None):
        t = cn.tile(shape or list(dram.shape), F32, tag=name)
        nc.sync.dma_start(out=t if rows is None else t[0:rows], in_=dram)
        return t

    wih0 = load("wih0", d_wih0, [D, 2, 4 * H])   # [128, dir, 512]
    whh0 = load("whh0", d_whh0, [H, 2, 4 * H])
    b0 = load("b0", d_b0, [H, 2, 4])
    wih1 = load("wih1", d_wih1, [H, 2, 2, 4 * H])  # [128, dir, kchunk, 512]
    whh1 = load("whh1", d_whh1, [H, 2, 4 * H])
    b1 = load("b1", d_b1, [H, 2, 4])
    w1t = load("w1t", d_w1t, [H, 2, 100])
    w2t = load("w2t", d_w2t, [H, 2, 100])
    fc1b = load("fc1b", d_fc1b, [128, 1], rows=100)
    w2aug = load("w2aug", d_w2aug, [128, 1], rows=101)
    tag_sb = load("temb", d_temb, [TT, PE_DIM])
    widx_t = cn.tile([128, nblk], I32, tag="widx")
    nc.sync.dma_start(out=widx_t, in_=d_widx)
    ident = cn.tile([128, 128], F32, tag="ident")
    make_identity(nc, ident)
    zrow = cn.tile([128, Bs], F32, tag="zrow")
    nc.vector.memset(zrow, 0.0)

    # ---- embedding ------------------------------------------------------
    emb_ctx = __import__("contextlib").ExitStack()
    xT = wk.tile([D, tok], F32, tag="xT")
    ps = emb_ctx.enter_context(tc.tile_pool(name="ps", bufs=1, space="PSUM"))
    ps_x = ps.tile([128, tok], F32, tag="psx")
    gat = emb_ctx.enter_context(tc.tile_pool(name="gat", bufs=2))
    for k in range(nblk):
        xw = gat.tile([128, WE], F32, tag="xw")
        nc.gpsimd.indirect_dma_start(
            out=xw[:], out_offset=None, in_=d_wemb[:],
            in_offset=bass.IndirectOffsetOnAxis(ap=widx_t[:, k:k + 1], axis=0))
        nc.tensor.transpose(out=ps_x[0:WE, k * 128:(k + 1) * 128], in_=xw[:],
                            identity=ident[:])
    nc.vector.tensor_copy(out=xT[0:WE, :], in_=ps_x[0:WE, :])
    # tag part: onehot matmul -> psum -> sbuf -> DMA into xT rows 100:128
    pidx_bc = wk.tile([TT, tok], F32, tag="pidxbc")
    nc.sync.dma_start(out=pidx_bc,
                      in_=bass.AP(tensor=d_pidx.tensor, offset=d_pidx.offset,
                                  ap=[[0, TT], [1, tok]]))
    iota_t = wk.tile([TT, tok], F32, tag="iota")
    nc.gpsimd.iota(iota_t, pattern=[[0, tok]], base=0, channel_multiplier=1,
                   allow_small_or_imprecise_dtypes=True)
    onehot = wk.tile([TT, tok], F32, tag="onehot")
    nc.vector.tensor_tensor(out=onehot, in0=iota_t, in1=pidx_bc, op=OP.is_equal)
    ps_tag = ps.tile([128, tok], F32, tag="pstag")
    nc.tensor.matmul(out=ps_tag[0:PE_DIM, :], lhsT=tag_sb[:], rhs=onehot[:],
                     start=True, stop=True)
    xp_sb = wk.tile([PE_DIM, tok], F32, tag="xpsb")
    nc.vector.tensor_copy(out=xp_sb, in_=ps_tag[0:PE_DIM, :])
    nc.sync.dma_start(out=xT[WE:D, :], in_=xp_sb)  # DMA: partition base 100 ok
    emb_ctx.close()

    # ---- LSTM layers ----------------------------------------------------
    lstm_ctx = __import__("contextlib").ExitStack()
    scr_pool = lstm_ctx.enter_context(tc.tile_pool(name="scr", bufs=2, space="PSUM"))
    u_pool = ctx.enter_context(tc.tile_pool(name="upool", bufs=2))
    z_pool = lstm_ctx.enter_context(tc.tile_pool(name="zpool", bufs=3, space="PSUM"))
    s_pool = ctx.enter_context(tc.tile_pool(name="spool", bufs=4))
    hs_pool = ctx.enter_context(tc.tile_pool(name="hspool", bufs=4))
    st_pool = ctx.enter_context(tc.tile_pool(name="stpool", bufs=1))

    def build_u(tag, wih_dir_aps, rhs_list, bias_col):
        # returns U sbuf tile [128, tok*8] cols (t, g, b); wih_dir_aps[r] is
        # the [128, 512] K-chunk lhsT AP matching rhs_list[r] [128, tok]
        U = u_pool.tile([128, tok * 4], F32, tag="U")
        for g in range(4):
            scr = scr_pool.tile([128, tok], F32, tag="scr")
            nchunk = len(rhs_list)
            for r in range(nchunk):
                nc.tensor.matmul(out=scr[:], lhsT=wih_dir_aps[r][:, g * H:(g + 1) * H],
                                 rhs=rhs_list[r], start=(r == 0), stop=(r == nchunk - 1))
            u_out = bass.AP(tensor=U.tensor, offset=U.offset + g * Bs,
                            ap=[U.ap[0][:], [4 * Bs, tok // Bs], [1, Bs]])
            nc.vector.tensor_scalar(out=u_out,
                                    in0=scr[:].rearrange("p (t b) -> p t b", b=Bs),
                                    scalar1=bias_col[:, g:g + 1], scalar2=None,
                                    op0=OP.add)
        return U

    def scan_layer(U_tiles, whh, lt):
        # U_tiles: per dir [128, tok*4]; whh: [128, dir, 512]; returns hs per dir
        hs = [hs_pool.tile([H, tok], F32, tag="hs") for _ in range(2)]
        cst = [st_pool.tile([H, Bs], F32, tag=f"c{lt}{d}") for d in range(2)]
        for d in range(2):
            nc.vector.memset(cst[d], 0.0)
        W = 4 * Bs
        for t in range(l):
            for d in range(2):
                p = t if d == 0 else l - 1 - t
                if t == 0:
                    rhs = zrow
                else:
                    pv = p - 1 if d == 0 else p + 1
                    rhs = hs[d][:, pv * Bs:(pv + 1) * Bs]
                z = z_pool.tile([128, W], F32, tag=f"z{d}")
                for g in range(4):
                    nc.tensor.matmul(out=z[:, g * Bs:(g + 1) * Bs],
                                     lhsT=whh[:, d, g * H:(g + 1) * H],
                                     rhs=rhs, start=True, stop=True)
                zs = s_pool.tile([128, W], F32, tag=f"zs{d}")
                nc.vector.tensor_tensor(out=zs, in0=z,
                                        in1=U_tiles[d][:, p * W:(p + 1) * W], op=OP.add)
                S = s_pool.tile([128, W], F32, tag=f"S{d}")
                nc.scalar.activation(S[:, 0:3 * Bs], zs[:, 0:3 * Bs], AF.Sigmoid)
                nc.scalar.activation(S[:, 3 * Bs:W], zs[:, 3 * Bs:W], AF.Tanh)
                tmp = s_pool.tile([128, Bs], F32, tag=f"tmp{d}")
                nc.vector.tensor_tensor(out=tmp, in0=S[:, 0:Bs], in1=S[:, 3 * Bs:W],
                                        op=OP.mult)
                nc.vector.tensor_tensor(out=cst[d], in0=S[:, Bs:2 * Bs], in1=cst[d],
                                        op=OP.mult)
                nc.vector.tensor_tensor(out=cst[d], in0=cst[d], in1=tmp, op=OP.add)
                thc = s_pool.tile([128, Bs], F32, tag=f"thc{d}")
                nc.scalar.activation(thc, cst[d], AF.Tanh)
                nc.vector.tensor_tensor(out=hs[d][:, p * Bs:(p + 1) * Bs],
                                        in0=S[:, 2 * Bs:3 * Bs], in1=thc, op=OP.mult)
        return hs

    U0 = [build_u("U0", [wih0[:, d, :]], [xT], b0[:, d, :]) for d in range(2)]
    hs0 = scan_layer(U0, whh0, 0)
    U1 = [build_u("U1", [wih1[:, d, 0, :], wih1[:, d, 1, :]], [hs0[0], hs0[1]],
                  b1[:, d, :]) for d in range(2)]
    hs1 = scan_layer(U1, whh1, 1)

    # ---- aT / cT --------------------------------------------------------
    lstm_ctx.close()
    ac_ps = ctx.enter_context(tc.tile_pool(name="acps", bufs=2, space="PSUM"))
    aT = wk.tile([128, tok], F32, tag="aT")
    cT = wk.tile([128, tok], F32, tag="cT")
    for which, wt, dst in (("a", w1t, aT), ("c", w2t, cT)):
        acp = ac_ps.tile([128, tok], F32, tag="ac")
        for r in range(2):
            nc.tensor.matmul(out=acp[0:100, :], lhsT=wt[:, r, :], rhs=hs1[r][:],
                             start=(r == 0), stop=(r == 1))
        if which == "a":
            nc.vector.tensor_copy(out=dst[0:100, :], in_=acp[0:100, :])
        else:
            nc.vector.tensor_scalar(out=dst[0:100, :], in0=acp[0:100, :],
                                    scalar1=fc1b[0:100, 0:1], scalar2=None, op0=OP.add)

    # ---- scorer ---------------------------------------------------------
    th_tiles = [wk.tile([128, GBLK * l], F32, tag=f"th{i}") for i in range(3)]
    for t_ in th_tiles:
        nc.vector.memset(t_[96:128, :], 1.0)
    mv_pool = ctx.enter_context(tc.tile_pool(name="mvps", bufs=3, space="PSUM"))
    stg_pool = ctx.enter_context(tc.tile_pool(name="stg", bufs=3))
    nmm = GBLK * l // 512
    for b in range(Bs):
        for blk in range(l // GBLK):
            i0 = blk * GBLK
            th = th_tiles[blk % 3]
            in_a = bass.AP(tensor=aT.tensor, offset=aT.offset + (i0 * Bs + b),
                           ap=[[aT.ap[0][0], 100], [Bs, GBLK], [0, l]])
            in_c = bass.AP(tensor=cT.tensor, offset=cT.offset + b,
                           ap=[[cT.ap[0][0], 100], [0, GBLK], [Bs, l]])
            nc.vector.tensor_tensor(
                out=th[0:100, :].rearrange("p (i j) -> p i j", i=GBLK),
                in0=in_a, in1=in_c, op=OP.add)
            nc.scalar.activation(th[0:100, :], th[0:100, :], AF.Tanh)
            mv = mv_pool.tile([128, 512], F32, tag="mv")
            for m in range(nmm):
                nc.tensor.matmul(out=mv[32 * m:32 * m + 1, :], lhsT=w2aug[0:101, 0:1],
                                 rhs=th[0:101, m * 512:(m + 1) * 512],
                                 start=True, stop=True, tile_position=(0, 32 * m))
            stage = stg_pool.tile([128, 512], F32, tag="stage")
            nc.vector.tensor_copy(out=stage, in_=mv)
            st_ap = bass.AP(tensor=stage.tensor, offset=stage.offset,
                            ap=[[32 * stage.ap[0][0], nmm], [1, 512]])
            out_ap = bass.AP(tensor=d_out.tensor,
                             offset=d_out.offset + b * l * l + i0 * l,
                             ap=[[512, nmm], [1, 512]])
            nc.sync.dma_start(out=out_ap, in_=st_ap)
    ctx.close()


def _prep_inputs(inputs, l=L):
    tok = l * Bs
    nblk = tok // 128
    widx = np.asarray(inputs["words_idx"], np.int64)[:, :l].astype(np.int32)
    pidx = np.asarray(inputs["pos_idx"], np.int64)[:, :l].astype(np.int32)
    wemb = np.ascontiguousarray(np.asarray(inputs["word_emb"], np.float32))
    temb = np.ascontiguousarray(np.asarray(inputs["tag_emb"], np.float32))

    per_layer = []
    for lw in (0, 1):
        dirs = []
        for d_ in (0, 1):
            dirs.append(_dir_weights(inputs[f"wih_l{lw}"][d_], inputs[f"whh_l{lw}"][d_],
                                     inputs[f"bih_l{lw}"][d_], inputs[f"bhh_l{lw}"][d_]))
        per_layer.append(dirs)
    # tile layouts: wih0 [D, dir, 512]; whh [H, dir, 512]; bias [H, dir, 4]
    wih0 = np.stack([per_layer[0][d][0] for d in range(2)], 1)
    whh0 = np.stack([per_layer[0][d][1] for d in range(2)], 1)
    b0 = np.stack([per_layer[0][d][2] for d in range(2)], 1)
    # wih1: per-dir [256, 512] -> [kchunk, H, 512]; want [H, dir, kchunk, 512]
    wih1 = np.stack([per_layer[1][d][0].reshape(2, H, 4 * H) for d in range(2)], 0)
    wih1 = np.ascontiguousarray(wih1.transpose(2, 0, 1, 3))
    whh1 = np.stack([per_layer[1][d][1] for d in range(2)], 1)
    b1 = np.stack([per_layer[1][d][2] for d in range(2)], 1)

    fc1w = np.asarray(inputs["fc1_w"], np.float32)
    dh = 2 * H
    w1t = np.ascontiguousarray(fc1w[:, :dh].T.reshape(2, H, 100).transpose(1, 0, 2))
    w2t = np.ascontiguousarray(fc1w[:, dh:].T.reshape(2, H, 100).transpose(1, 0, 2))
    fc1b = np.asarray(inputs["fc1_b"], np.float32).reshape(100, 1)
    w2aug = np.concatenate([np.asarray(inputs["fc2_w"], np.float32).reshape(100, 1),
                            np.asarray(inputs["fc2_b"], np.float32).reshape(1, 1)], 0)

    def fix(a):
        return np.ascontiguousarray(a.astype(np.float32))

    in_maps = []
    for core in range(NCORES):
        rows = slice(core * Bs, (core + 1) * Bs)
        wi = widx[rows]   # [Bs, l]
        pi = pidx[rows]
        wflat = np.ascontiguousarray(wi.T).reshape(tok)   # n = t*Bs + b
        pflat = np.ascontiguousarray(pi.T).reshape(tok)
        in_maps.append(dict(
            widx=np.ascontiguousarray(wflat.reshape(nblk, 128).T),
            pidx=pflat.reshape(1, tok).astype(np.float32),
            wemb=wemb, temb=temb,
            wih0=fix(wih0), whh0=fix(whh0), b0=fix(b0),
            wih1=fix(wih1), whh1=fix(whh1), b1=fix(b1),
            w1t=fix(w1t), w2t=fix(w2t), fc1b=fix(fc1b), w2aug=fix(w2aug),
        ))
    return in_maps


def kernel(**inputs):
    ml = int(inputs.get("max_length", L))
    assert ml == L, f"kernel hardcodes max_length={L}, got {ml}"
    if "nc" not in _CACHE:
        _CACHE["nc"] = _build()
    nc = _CACHE["nc"]
    in_maps = _prep_inputs(inputs)
    res = bass_utils.run_bass_kernel_spmd(nc, in_maps, core_ids=list(range(NCORES)))
    out = np.empty((B, L, L), np.float32)
    for core in range(NCORES):
        out[core * Bs:(core + 1) * Bs] = res.results[core]["scores"]
    return np.ascontiguousarray(out.transpose(1, 0, 2)[..., None])
